# revision 26
# baseline (speedup 1.0000x reference)
"""Trainium2 Bass kernel for nn_Attention_6133213298828 (v2).

Batch-parallel multi-head attention with relative-position logits, forward
only. Data-parallel over 8 NeuronCores (batch dim); weights replicated.

v2 design (per core: 2048 batches = 16 chunks x 128 batches x 17 tokens):
  - fp16 I/O: x is pre-cast to fp16 on host, y is produced fp16 and upcast
    on host -> halves HBM traffic.
  - x^T comes straight from HBM via two xbar DMA-transposes per chunk
    (no on-chip transposes of x, no xnat tile).
  - Attention runs on OVERLAPPED 128-token key/query groups with stride
    119 (= 7 batches x 17). Groups own 119 "real" queries; the 9 padded
    queries/keys belong to the following batch and are masked / discarded.
    This makes every hot matmul stationary exactly 128 columns (FWL).
  - Scores per (g,h): TWO accumulating matmuls into one fp32 psum tile
    [128k x 128q]: (1) mask+rel matmul: static stationary eml90 pattern
    block (26 rows at partition 32h) x frm moving (rel projections +
    static query-mask rows), then (2) K^T x Q (64 rows). Shallow matmul
    first so the deeper one's per-element drain always lands later.
  - Softmax: bounded logits -> exp without max-subtraction; denominator
    from a ones column in V; normalization folded into one
    tensor_tensor multiply per AV psum tile with a broadcast reciprocal.
  - AV, V-proj, out-proj stationaries all 128 columns (overlapped groups).
"""

import numpy as np

DIM, OUT_DIM, H, V, B = 192, 192, 3, 17, 16384
DK = DIM // H
NCORES = 8
BC = B // NCORES          # batches per core
NB = 128                  # batches per chunk
NCHUNK = BC // NB         # 16
TC = NB * V               # 2176 tokens per chunk
TOK = BC * V              # 34816 tokens per core
GS = 119                  # group stride (7 batches x 17)
G = 19                    # groups per chunk (18 full-stride + tail)
MASKC = float(np.sqrt(30.0))
SCALE = DIM ** -0.5

def _gdims(g):
    """(token_start, n_keys, n_queries) for group g within a chunk."""
    t0 = g * GS
    n = min(GS, TC - t0)
    return t0, n, n

_CACHED = {}


def _build_host_constants(W_qkv, b_qkv, key_rel, key_rel_diag, W_out, b_out):
    f16 = np.float16
    scale = np.float32(SCALE)

    # Per-head slabs: slab h = [q_h * scale (64) | k_h (64)].
    qs = W_qkv[:, 0:DIM] * scale
    kk = W_qkv[:, DIM:2 * DIM]
    wqk = np.concatenate(
        [np.concatenate([qs[:, h * DK:(h + 1) * DK],
                         kk[:, h * DK:(h + 1) * DK]], axis=1)
         for h in range(H)], axis=1)
    wv = W_qkv[:, 2 * DIM:3 * DIM]

    # KRABS[i, j] = relative key vector seen by query position i at key
    # position j (diag on j == i).
    kr = key_rel.reshape(V, V - 1, DK)
    KRABS = np.zeros((V, V, DK), np.float32)
    for i in range(V):
        for j in range(V):
            KRABS[i, j] = key_rel_diag[0] if j == i else kr[i, j - (j > i)]

    # wrel[i]: (192, 96); cols 32h + j (j < 17) = scale * W_qh @ KRABS[i, j]
    wrel = np.zeros((V, DIM, 96), np.float32)
    for h in range(H):
        wq_h = W_qkv[:, h * DK:(h + 1) * DK]
        proj = np.einsum('dk,ijk->dij', wq_h, KRABS) * scale
        for i in range(V):
            wrel[i, :, 32 * h:32 * h + 17] = proj[:, i, :]

    # Static key-side rows of kf (kf rows 64..88): per TOKEN t, viewed as
    # group-local l = t mod 119 (also serves the overlap use l+119 of the
    # previous group -- consistent because 119 = 7*17 and overlap keys are
    # masked purely by the -30 ones-row, there is no a=7 indicator).
    # rows 0-16: one-hot(t mod 17 == r); 17-23: MASKC*((t mod 119)//17 == a);
    # row 24: ones.
    t = np.arange(TC)
    bq = (t % GS) // V
    emlk = np.zeros((25, TC), np.float32)
    for r in range(V):
        emlk[r] = (t % V == r)
    for a in range(7):
        emlk[17 + a] = MASKC * (bq == a)
    emlk[24] = 1.0

    # Static query-side rows of qf (qf rows 81..88):
    # rows 0-6: MASKC*((t mod 119)//17 == b); row 7: -30.
    maskq = np.zeros((8, TC), np.float32)
    for b in range(7):
        maskq[b] = MASKC * (bq == b)
    maskq[7] = -30.0

    consts = {
        "wqk0": wqk[0:128].astype(f16),
        "wqk1": wqk[128:192].astype(f16),
        "wv0": wv[0:128].astype(f16),
        "wv1": wv[128:192].astype(f16),
        "wout0": W_out[0:128].astype(f16),
        "wout1": W_out[128:192].astype(f16),
        "wrel0": wrel[:, 0:128, :].reshape(V * 128, 96).astype(f16),
        "wrel1": wrel[:, 128:192, :].reshape(V * 64, 96).astype(f16),
        "emlk": emlk.astype(f16),
        "maskq": maskq.astype(f16),
        "ident": np.eye(128, dtype=f16),
    }
    return consts


def _build_bass():
    import concourse.bacc as bacc
    import concourse.mybir as mybir
    from concourse import tile

    f16 = mybir.dt.float16
    f32 = mybir.dt.float32
    EXP = mybir.ActivationFunctionType.Exp

    nc = bacc.Bacc(None, target_bir_lowering=False)

    # x arrives pre-transposed per chunk: [NCHUNK * 192, TC], chunk c rows
    # c*192 .. c*192+192 hold x[chunk c]^T (feature-major).
    x_in = nc.declare_dram_parameter("x", [NCHUNK * DIM, TC], f16, isOutput=False)
    dp = lambda name, shape: nc.declare_dram_parameter(name, list(shape), f16, isOutput=False)
    wqk0_d = dp("wqk0", (128, 384)); wqk1_d = dp("wqk1", (64, 384))
    wv0_d = dp("wv0", (128, 192)); wv1_d = dp("wv1", (64, 192))
    wout0_d = dp("wout0", (128, 192)); wout1_d = dp("wout1", (64, 192))
    wrel0_d = dp("wrel0", (V * 128, 96)); wrel1_d = dp("wrel1", (V * 64, 96))
    emlk_d = dp("emlk", (25, TC)); maskq_d = dp("maskq", (8, TC))
    ident_d = dp("ident", (128, 128))
    y_out = nc.declare_dram_parameter("y", [TOK, DIM], f16, isOutput=True)

    NT512 = [(0, 512), (512, 512), (1024, 512), (1536, 512), (2048, 128)]

    with tile.TileContext(nc) as tc:
        with tc.sbuf_pool(name="wpool", bufs=1) as wp, \
             tc.sbuf_pool(name="work", bufs=2) as sp, \
             tc.psum_pool(name="ps", bufs=4) as ps, \
             tc.psum_pool(name="pst", bufs=2) as pst:

            # ---- persistent weights ----
            wqk0 = wp.tile([128, 384], f16); nc.sync.dma_start(out=wqk0[:], in_=wqk0_d[:])
            wqk1 = wp.tile([64, 384], f16); nc.sync.dma_start(out=wqk1[:], in_=wqk1_d[:])
            wv0 = wp.tile([128, 192], f16); nc.sync.dma_start(out=wv0[:], in_=wv0_d[:])
            wv1 = wp.tile([64, 192], f16); nc.sync.dma_start(out=wv1[:], in_=wv1_d[:])
            wout0 = wp.tile([128, 192], f16); nc.sync.dma_start(out=wout0[:], in_=wout0_d[:])
            wout1 = wp.tile([64, 192], f16); nc.sync.dma_start(out=wout1[:], in_=wout1_d[:])
            wrel0 = wp.tile([128, V * 96], f16)
            nc.sync.dma_start(out=wrel0[:].rearrange("p (i j) -> p i j", j=96),
                              in_=wrel0_d[:].rearrange("(i p) j -> p i j", p=128))
            wrel1 = wp.tile([64, V * 96], f16)
            nc.sync.dma_start(out=wrel1[:].rearrange("p (i j) -> p i j", j=96),
                              in_=wrel1_d[:].rearrange("(i p) j -> p i j", p=64))
            ident = wp.tile([128, 128], f16); nc.sync.dma_start(out=ident[:], in_=ident_d[:])

            for c in range(NCHUNK):
                r0 = c * TC
                # ---- x^T loaded directly (host pre-transposed) ----
                xt0 = sp.tile([128, TC], f16, tag="xt0")
                xt1t = sp.tile([64, TC], f16, tag="xt1")
                nc.sync.dma_start(out=xt0[:], in_=x_in[c * DIM:c * DIM + 128, :])
                nc.scalar.dma_start(out=xt1t[:], in_=x_in[c * DIM + 128:(c + 1) * DIM, :])
                xt1 = xt1t[0:64]

                # ---- QK projections -> per-head qf/kf tiles ----
                # qf_h rows: 0-63 q_h*scale, 64-80 rel logits, 81-87 static
                # query-mask rows, 88 = -30.  kf_h rows: 0-63 k_h, 64-88
                # static key-side pattern (pos one-hots, batch indicators,
                # ones).  One fused 89-deep matmul per (g,h) computes
                # scores + rel + mask in a single pass.
                qf = [sp.tile([89, TC], f16, tag=f"qf{h}", name=f"qf{h}")
                      for h in range(H)]
                kf = [sp.tile([89, TC], f16, tag=f"kf{h}", name=f"kf{h}")
                      for h in range(H)]
                for h in range(H):
                    nc.sync.dma_start(out=kf[h][64:89, :], in_=emlk_d[:])
                    nc.scalar.dma_start(out=qf[h][81:89, :], in_=maskq_d[:])
                # evict whole [128,512] psum once per tile into a scratch
                # slab; the q/k split into qf/kf runs on the (idle) DMA
                # engines as SBUF->SBUF copies.
                qks = [sp.tile([128, TC], f16, tag=f"qks{h}", name=f"qks{h}")
                       for h in range(H)]
                for m in range(3):
                    for ni, (n0, nw) in enumerate(NT512):
                        pq = ps.tile([128, 512], f32, tag="ps32")
                        nc.tensor.matmul(pq[:, 0:nw], wqk0[:, m * 128:(m + 1) * 128],
                                         xt0[:, n0:n0 + nw], start=True, stop=False)
                        nc.tensor.matmul(pq[:, 0:nw], wqk1[:, m * 128:(m + 1) * 128],
                                         xt1[:, n0:n0 + nw], start=False, stop=True)
                        if (m * 5 + ni) % 2 == 0:
                            nc.vector.tensor_copy(qks[m][:, n0:n0 + nw], pq[:, 0:nw])
                        else:
                            nc.scalar.copy(qks[m][:, n0:n0 + nw], pq[:, 0:nw])
                for h in range(H):
                    nc.gpsimd.dma_start(out=qf[h][0:64, :], in_=qks[h][0:64, :])
                    nc.gpsimd.dma_start(out=kf[h][0:64, :], in_=qks[h][64:128, :])

                # ---- rel projections -> qf rows 64..80 ----
                xt0v = xt0[:].rearrange("p (b v) -> p b v", v=V)
                xt1v = xt1.rearrange("p (b v) -> p b v", v=V)
                relsc = sp.tile([96, TC], f16, tag="relsc")
                relv = relsc[:].rearrange("p (b v) -> p b v", v=V)
                for ip in range(5):          # packs of 4 positions
                    n = min(4, V - ip * 4)
                    pr = ps.tile([96, 512], f32, tag="ps32")
                    for u in range(n):
                        i = ip * 4 + u
                        nc.tensor.matmul(pr[:, u * 128:u * 128 + 128],
                                         wrel0[:, i * 96:(i + 1) * 96],
                                         xt0v[:, :, i], start=True, stop=False)
                        nc.tensor.matmul(pr[:, u * 128:u * 128 + 128],
                                         wrel1[:, i * 96:(i + 1) * 96],
                                         xt1v[:, :, i], start=False, stop=True)
                    # one eviction per pack into the scratch (token-ordered);
                    # per-head row split runs on the DMA engines below
                    src = pr[:].rearrange("p (u b) -> p b u", b=128)
                    dst = relv[:, :, ip * 4:ip * 4 + n]
                    if ip % 2 == 0:
                        nc.vector.tensor_copy(dst, src[:, :, 0:n])
                    else:
                        nc.scalar.copy(dst, src[:, :, 0:n])
                for h in range(H):
                    nc.gpsimd.dma_start(out=qf[h][64:81, :],
                                        in_=relsc[32 * h:32 * h + 17, :])

                # ---- scores: one fused 89-deep matmul per (g,h), exp ----
                attn = sp.tile([119, 57 * 128], f16, tag="attn")
                for pk in range(15):         # packs of 4 (g,h) tiles; 57=14*4+1
                    n = min(4, 57 - pk * 4)
                    pd = ps.tile([119, 512], f32, tag="ps32")
                    for u in range(n):
                        idx = pk * 4 + u
                        g, h = divmod(idx, H)
                        t0, kn, qn = _gdims(g)
                        o = u * 128
                        nc.tensor.matmul(pd[0:kn, o:o + qn],
                                         kf[h][:, t0:t0 + kn],
                                         qf[h][:, t0:t0 + qn],
                                         start=True, stop=True)
                    nc.scalar.activation(attn[:, pk * 512:pk * 512 + n * 128],
                                         pd[:, 0:n * 128], EXP)

                # ---- V projection (overlapped 128-token groups) ----
                vt = sp.tile([119, G * 195], f16, tag="vt")
                nc.gpsimd.memset(
                    vt[:].rearrange("p (g hh c) -> p g hh c", hh=3, c=65)[:, :, :, 64:65],
                    1.0)
                for gp in range(10):         # packs of 2 groups
                    n = min(2, G - gp * 2)
                    pv = ps.tile([119, 384], f32, tag="ps32")
                    for u in range(n):
                        g = gp * 2 + u
                        t0, kn, qn = _gdims(g)
                        gk = slice(t0, t0 + kn)
                        nc.tensor.matmul(pv[0:kn, u * 192:u * 192 + 192],
                                         xt0[:, gk], wv0[:], start=True, stop=False)
                        nc.tensor.matmul(pv[0:kn, u * 192:u * 192 + 192],
                                         xt1[:, gk], wv1[:], start=False, stop=True)
                    g0 = gp * 2
                    src = pv[:].rearrange("p (g hh c) -> p g hh c", hh=3, c=64)[:, 0:n]
                    dst = vt[:].rearrange("p (g hh c) -> p g hh c", hh=3, c=65)[
                        :, g0:g0 + n, :, 0:64]
                    if gp % 2 == 0:
                        nc.vector.tensor_copy(dst, src)
                    else:
                        nc.scalar.copy(dst, src)

                # ---- attention @ V (+denominator), normalize via TT ----
                avout = sp.tile([119, G * 192], f16, tag="avout")
                recip = sp.tile([119, G * H], f32, tag="recip")
                vtv = vt[:].rearrange("p (g c) -> p g c", c=195)
                for gp in range(10):         # packs of 2 groups
                    n = min(2, G - gp * 2)
                    pa = ps.tile([119, 390], f32, tag="ps32")
                    for u in range(n):
                        g = gp * 2 + u
                        t0, kn, qn = _gdims(g)
                        for h in range(H):
                            idx = g * H + h
                            nc.tensor.matmul(
                                pa[0:qn, u * 195 + 65 * h:u * 195 + 65 * h + 65],
                                attn[0:kn, idx * 128:idx * 128 + qn],
                                vtv[0:kn, g, 65 * h:65 * h + 65],
                                start=True, stop=True)
                    g0 = gp * 2
                    pav = pa[:].rearrange("p (g hh c) -> p g hh c", hh=3, c=65)
                    nc.vector.reciprocal(
                        recip[:, g0 * H:(g0 + n) * H].rearrange(
                            "p (g hh) -> p g hh", hh=3),
                        pav[:, 0:n, :, 64])
                    rb = recip[:, g0 * H:(g0 + n) * H].rearrange(
                        "p (g hh) -> p g hh", hh=3).unsqueeze(3).broadcast_to(
                        (119, n, 3, 64))
                    nc.vector.tensor_mul(
                        avout[:].rearrange("p (g hh c) -> p g hh c", hh=3, c=64)[
                            :, g0:g0 + n],
                        pav[:, 0:n, :, 0:64], rb)

                # ---- transpose avout to feature-major ----
                aot0 = sp.tile([128, G * 128], f16, tag="aot0")
                aot1 = sp.tile([64, G * 128], f16, tag="aot1")
                for gp in range(5):          # packs of 4 groups
                    n = min(4, G - gp * 4)
                    pta = pst.tile([128, 512], f16, tag="pst")
                    ptb = pst.tile([64, 512], f16, tag="pstb")
                    for u in range(n):
                        g = gp * 4 + u
                        t0, kn, qn = _gdims(g)
                        nc.tensor.transpose(pta[:, u * 128:u * 128 + qn],
                                            avout[0:qn, g * 192:g * 192 + 128],
                                            ident[0:qn, 0:qn])
                        nc.tensor.transpose(ptb[:, u * 128:u * 128 + qn],
                                            avout[0:qn, g * 192 + 128:g * 192 + 192],
                                            ident[0:qn, 0:qn])
                    cs = slice(gp * 512, gp * 512 + n * 128)
                    nc.vector.tensor_copy(aot0[:, cs], pta[:, 0:n * 128])
                    nc.scalar.copy(aot1[:, cs], ptb[:, 0:n * 128])

                # ---- output projection ----
                fin = sp.tile([119, G * 192], f16, tag="fin")
                for gp in range(10):
                    n = min(2, G - gp * 2)
                    po = ps.tile([119, 384], f32, tag="ps32")
                    for u in range(n):
                        g = gp * 2 + u
                        t0, kn, qn = _gdims(g)
                        nc.tensor.matmul(po[0:qn, u * 192:u * 192 + 192],
                                         aot0[:, g * 128:g * 128 + qn],
                                         wout0[:], start=True, stop=False)
                        nc.tensor.matmul(po[0:qn, u * 192:u * 192 + 192],
                                         aot1[:, g * 128:g * 128 + qn],
                                         wout1[:], start=False, stop=True)
                    g0 = gp * 2
                    dst = fin[:, g0 * 192:(g0 + n) * 192]
                    if gp % 2 == 0:
                        nc.scalar.copy(dst, po[:, 0:n * 192])
                    else:
                        nc.vector.tensor_copy(dst, po[:, 0:n * 192])

                # ---- store ----
                nc.sync.dma_start(
                    out=y_out[r0:r0 + 18 * GS, :].rearrange("(g p) d -> p g d", p=GS),
                    in_=fin[:].rearrange("p (g d) -> p g d", d=192)[:, 0:18, :])
                nc.sync.dma_start(
                    out=y_out[r0 + 18 * GS:r0 + TC, :],
                    in_=fin[0:34, 18 * 192:19 * 192])

    nc.finalize()
    return nc


def kernel(x, W_qkv, b_qkv, key_rel, key_rel_diag, W_out, b_out):
    from concourse.bass_utils import run_bass_kernel_spmd

    # pre-transpose per chunk on host: (core, chunk, TC, DIM) -> (core, chunk, DIM, TC)
    xt = np.ascontiguousarray(
        np.asarray(x, dtype=np.float16).reshape(NCORES, NCHUNK, TC, DIM)
        .transpose(0, 1, 3, 2)).reshape(NCORES, NCHUNK * DIM, TC)
    consts = _build_host_constants(
        np.asarray(W_qkv, np.float32), np.asarray(b_qkv, np.float32),
        np.asarray(key_rel, np.float32), np.asarray(key_rel_diag, np.float32),
        np.asarray(W_out, np.float32), np.asarray(b_out, np.float32))

    if "nc" not in _CACHED:
        _CACHED["nc"] = _build_bass()
    nc = _CACHED["nc"]

    in_maps = [dict(consts, x=xt[k]) for k in range(NCORES)]
    res = run_bass_kernel_spmd(nc, in_maps, core_ids=list(range(NCORES)))
    _CACHED["last_result"] = res
    out = np.stack([res.results[k]["y"] for k in range(NCORES)], axis=0)
    return out.reshape(B, V, DIM).astype(np.float32)


# revision 29
# speedup vs baseline: 1.0463x; 1.0463x over previous
"""Trainium2 Bass kernel for nn_Attention_6133213298828 (v2).

Batch-parallel multi-head attention with relative-position logits, forward
only. Data-parallel over 8 NeuronCores (batch dim); weights replicated.

v2 design (per core: 2048 batches = 16 chunks x 128 batches x 17 tokens):
  - fp16 I/O: x is pre-cast to fp16 on host, y is produced fp16 and upcast
    on host -> halves HBM traffic.
  - x^T comes straight from HBM via two xbar DMA-transposes per chunk
    (no on-chip transposes of x, no xnat tile).
  - Attention runs on OVERLAPPED 128-token key/query groups with stride
    119 (= 7 batches x 17). Groups own 119 "real" queries; the 9 padded
    queries/keys belong to the following batch and are masked / discarded.
    This makes every hot matmul stationary exactly 128 columns (FWL).
  - Scores per (g,h): TWO accumulating matmuls into one fp32 psum tile
    [128k x 128q]: (1) mask+rel matmul: static stationary eml90 pattern
    block (26 rows at partition 32h) x frm moving (rel projections +
    static query-mask rows), then (2) K^T x Q (64 rows). Shallow matmul
    first so the deeper one's per-element drain always lands later.
  - Softmax: bounded logits -> exp without max-subtraction; denominator
    from a ones column in V; normalization folded into one
    tensor_tensor multiply per AV psum tile with a broadcast reciprocal.
  - AV, V-proj, out-proj stationaries all 128 columns (overlapped groups).
"""

import numpy as np

DIM, OUT_DIM, H, V, B = 192, 192, 3, 17, 16384
DK = DIM // H
NCORES = 8
BC = B // NCORES          # batches per core
NB = 128                  # batches per chunk
NCHUNK = BC // NB         # 16
TC = NB * V               # 2176 tokens per chunk
TOK = BC * V              # 34816 tokens per core
GS = 119                  # group stride (7 batches x 17)
G = 19                    # groups per chunk (18 full-stride + tail)
MASKC = float(np.sqrt(30.0))
SCALE = DIM ** -0.5

def _gdims(g):
    """(token_start, n_keys, n_queries) for group g within a chunk."""
    t0 = g * GS
    n = min(GS, TC - t0)
    return t0, n, n

_CACHED = {}


def _build_host_constants(W_qkv, b_qkv, key_rel, key_rel_diag, W_out, b_out):
    f16 = np.float16
    scale = np.float32(SCALE)

    # Per-head slabs: slab h = [q_h * scale (64) | k_h (64)].
    qs = W_qkv[:, 0:DIM] * scale
    kk = W_qkv[:, DIM:2 * DIM]
    wqk = np.concatenate(
        [np.concatenate([qs[:, h * DK:(h + 1) * DK],
                         kk[:, h * DK:(h + 1) * DK]], axis=1)
         for h in range(H)], axis=1)
    wv = W_qkv[:, 2 * DIM:3 * DIM]

    # KRABS[i, j] = relative key vector seen by query position i at key
    # position j (diag on j == i).
    kr = key_rel.reshape(V, V - 1, DK)
    KRABS = np.zeros((V, V, DK), np.float32)
    for i in range(V):
        for j in range(V):
            KRABS[i, j] = key_rel_diag[0] if j == i else kr[i, j - (j > i)]

    # wrel[i]: (192, 128); cols 32h + j (j < 17) = scale * W_qh @ KRABS[i, j]
    # (padded to 128 columns so LDWEIGHTS qualifies for fast weight load)
    wrel = np.zeros((V, DIM, 128), np.float32)
    for h in range(H):
        wq_h = W_qkv[:, h * DK:(h + 1) * DK]
        proj = np.einsum('dk,ijk->dij', wq_h, KRABS) * scale
        for i in range(V):
            wrel[i, :, 32 * h:32 * h + 17] = proj[:, i, :]

    # Static key-side rows of kf (kf rows 64..88): per TOKEN t, viewed as
    # group-local l = t mod 119 (also serves the overlap use l+119 of the
    # previous group -- consistent because 119 = 7*17 and overlap keys are
    # masked purely by the -30 ones-row, there is no a=7 indicator).
    # rows 0-16: one-hot(t mod 17 == r); 17-23: MASKC*((t mod 119)//17 == a);
    # row 24: ones.
    t = np.arange(TC)
    bq = (t % GS) // V
    emlk = np.zeros((25, TC), np.float32)
    for r in range(V):
        emlk[r] = (t % V == r)
    for a in range(7):
        emlk[17 + a] = MASKC * (bq == a)
    emlk[24] = 1.0

    # Static query-side rows of qf (qf rows 81..88):
    # rows 0-6: MASKC*((t mod 119)//17 == b); row 7: -30.
    maskq = np.zeros((8, TC), np.float32)
    for b in range(7):
        maskq[b] = MASKC * (bq == b)
    maskq[7] = -30.0

    consts = {
        "wqk0": wqk[0:128].astype(f16),
        "wqk1": wqk[128:192].astype(f16),
        "wv0": wv[0:128].astype(f16),
        "wv1": wv[128:192].astype(f16),
        "wout0": W_out[0:128].astype(f16),
        "wout1": W_out[128:192].astype(f16),
        "wrel0": wrel[:, 0:128, :].reshape(V * 128, 128).astype(f16),
        "wrel1": wrel[:, 128:192, :].reshape(V * 64, 128).astype(f16),
        "emlk": emlk.astype(f16),
        "maskq": maskq.astype(f16),
        "ident": np.eye(128, dtype=f16),
    }
    return consts


def _build_bass():
    import concourse.bacc as bacc
    import concourse.mybir as mybir
    from concourse import tile

    f16 = mybir.dt.float16
    f32 = mybir.dt.float32
    EXP = mybir.ActivationFunctionType.Exp

    nc = bacc.Bacc(None, target_bir_lowering=False)

    # x arrives pre-transposed per chunk: [NCHUNK * 192, TC], chunk c rows
    # c*192 .. c*192+192 hold x[chunk c]^T (feature-major).
    x_in = nc.declare_dram_parameter("x", [NCHUNK * DIM, TC], f16, isOutput=False)
    dp = lambda name, shape: nc.declare_dram_parameter(name, list(shape), f16, isOutput=False)
    wqk0_d = dp("wqk0", (128, 384)); wqk1_d = dp("wqk1", (64, 384))
    wv0_d = dp("wv0", (128, 192)); wv1_d = dp("wv1", (64, 192))
    wout0_d = dp("wout0", (128, 192)); wout1_d = dp("wout1", (64, 192))
    wrel0_d = dp("wrel0", (V * 128, 128)); wrel1_d = dp("wrel1", (V * 64, 128))
    emlk_d = dp("emlk", (25, TC)); maskq_d = dp("maskq", (8, TC))
    ident_d = dp("ident", (128, 128))
    y_out = nc.declare_dram_parameter("y", [TOK, DIM], f16, isOutput=True)

    NT512 = [(0, 512), (512, 512), (1024, 512), (1536, 512), (2048, 128)]

    with tile.TileContext(nc) as tc:
        with tc.sbuf_pool(name="wpool", bufs=1) as wp, \
             tc.sbuf_pool(name="work", bufs=2) as sp, \
             tc.psum_pool(name="ps", bufs=4) as ps, \
             tc.psum_pool(name="pst", bufs=2) as pst:

            # ---- persistent weights ----
            wqk0 = wp.tile([128, 384], f16); nc.sync.dma_start(out=wqk0[:], in_=wqk0_d[:])
            wqk1 = wp.tile([64, 384], f16); nc.sync.dma_start(out=wqk1[:], in_=wqk1_d[:])
            wv0 = wp.tile([128, 192], f16); nc.sync.dma_start(out=wv0[:], in_=wv0_d[:])
            wv1 = wp.tile([64, 192], f16); nc.sync.dma_start(out=wv1[:], in_=wv1_d[:])
            wout0 = wp.tile([128, 192], f16); nc.sync.dma_start(out=wout0[:], in_=wout0_d[:])
            wout1 = wp.tile([64, 192], f16); nc.sync.dma_start(out=wout1[:], in_=wout1_d[:])
            wrel0 = wp.tile([128, V * 128], f16)
            nc.sync.dma_start(out=wrel0[:].rearrange("p (i j) -> p i j", j=128),
                              in_=wrel0_d[:].rearrange("(i p) j -> p i j", p=128))
            wrel1 = wp.tile([64, V * 128], f16)
            nc.sync.dma_start(out=wrel1[:].rearrange("p (i j) -> p i j", j=128),
                              in_=wrel1_d[:].rearrange("(i p) j -> p i j", p=64))
            ident = wp.tile([128, 128], f16); nc.sync.dma_start(out=ident[:], in_=ident_d[:])

            for c in range(NCHUNK):
                r0 = c * TC
                # ---- x^T loaded directly (host pre-transposed) ----
                xt0 = sp.tile([128, TC], f16, tag="xt0")
                xt1t = sp.tile([64, TC], f16, tag="xt1")
                nc.sync.dma_start(out=xt0[:], in_=x_in[c * DIM:c * DIM + 128, :])
                nc.scalar.dma_start(out=xt1t[:], in_=x_in[c * DIM + 128:(c + 1) * DIM, :])
                xt1 = xt1t[0:64]

                # ---- QK projections -> per-head qf/kf tiles ----
                # qf_h rows: 0-63 q_h*scale, 64-80 rel logits, 81-87 static
                # query-mask rows, 88 = -30.  kf_h rows: 0-63 k_h, 64-88
                # static key-side pattern (pos one-hots, batch indicators,
                # ones).  One fused 89-deep matmul per (g,h) computes
                # scores + rel + mask in a single pass.
                qf = [sp.tile([89, TC], f16, tag=f"qf{h}", name=f"qf{h}")
                      for h in range(H)]
                kf = [sp.tile([89, TC], f16, tag=f"kf{h}", name=f"kf{h}")
                      for h in range(H)]
                for h in range(H):
                    nc.sync.dma_start(out=kf[h][64:89, :], in_=emlk_d[:])
                    nc.scalar.dma_start(out=qf[h][81:89, :], in_=maskq_d[:])
                # evict whole [128,512] psum once per tile into a scratch
                # slab; the q/k split into qf/kf runs on the (idle) DMA
                # engines as SBUF->SBUF copies.
                qks = [sp.tile([128, TC], f16, tag=f"qks{h}", name=f"qks{h}")
                       for h in range(H)]
                qk_tiles = [(m, n0, nw) for m in range(3) for n0, nw in NT512]
                for w0 in range(0, 15, 2):
                    pair = qk_tiles[w0:w0 + 2]
                    pqs = []
                    for _ in pair:
                        pq = ps.tile([128, 512], f32, tag="ps32", name="pq")
                        pqs.append(pq)
                    for (m, n0, nw), pq in zip(pair, pqs):
                        nc.tensor.matmul(pq[:, 0:nw], wqk0[:, m * 128:(m + 1) * 128],
                                         xt0[:, n0:n0 + nw], start=True, stop=False)
                    for (m, n0, nw), pq in zip(pair, pqs):
                        nc.tensor.matmul(pq[:, 0:nw], wqk1[:, m * 128:(m + 1) * 128],
                                         xt1[:, n0:n0 + nw], start=False, stop=True)
                    for j, ((m, n0, nw), pq) in enumerate(zip(pair, pqs)):
                        if (w0 + j) % 2 == 0:
                            nc.vector.tensor_copy(qks[m][:, n0:n0 + nw], pq[:, 0:nw])
                        else:
                            nc.scalar.copy(qks[m][:, n0:n0 + nw], pq[:, 0:nw])
                for h in range(H):
                    nc.gpsimd.dma_start(out=qf[h][0:64, :], in_=qks[h][0:64, :])
                    nc.gpsimd.dma_start(out=kf[h][0:64, :], in_=qks[h][64:128, :])

                # ---- rel projections -> qf rows 64..80 ----
                xt0v = xt0[:].rearrange("p (b v) -> p b v", v=V)
                xt1v = xt1.rearrange("p (b v) -> p b v", v=V)
                relsc = sp.tile([96, TC], f16, tag="relsc")
                relv = relsc[:].rearrange("p (b v) -> p b v", v=V)
                for ip in range(5):          # packs of 4 positions, 2 banks
                    n = min(4, V - ip * 4)
                    nbk = 1 if n <= 2 else 2
                    prs = []
                    for half in range(nbk):
                        pr = ps.tile([128, 512], f32, tag="ps32", name="pr")
                        prs.append(pr)
                    # positions: bank A holds local 0,1; bank B holds 2,3
                    locs = [(u // 2, u % 2) for u in range(n)]
                    started = [False] * nbk
                    for w, (wr0, wr1, xv) in enumerate(
                            [(wrel0, None, xt0v), (None, wrel1, xt1v)]):
                        for u in range(n):
                            bk, s = locs[u]
                            i = ip * 4 + u
                            wr = wr0 if w == 0 else wr1
                            nc.tensor.matmul(prs[bk][:, s * 128:s * 128 + 128],
                                             wr[:, i * 128:(i + 1) * 128],
                                             xv[:, :, i],
                                             start=(w == 0 and not started[bk]),
                                             stop=(w == 1 and u == max(
                                                 uu for uu in range(n)
                                                 if locs[uu][0] == bk)),
                                             skip_group_check=True)
                            if w == 0:
                                started[bk] = True
                    for bk in range(nbk):
                        us = [u for u in range(n) if locs[u][0] == bk]
                        srcv = prs[bk][:].rearrange("p (u b) -> p b u", b=128)
                        dst = relv[:, :, ip * 4 + us[0]:ip * 4 + us[-1] + 1]
                        if (ip + bk) % 2 == 0:
                            nc.vector.tensor_copy(dst, srcv[0:96, :, 0:len(us)])
                        else:
                            nc.scalar.copy(dst, srcv[0:96, :, 0:len(us)])
                for h in range(H):
                    nc.gpsimd.dma_start(out=qf[h][64:81, :],
                                        in_=relsc[32 * h:32 * h + 17, :])

                # ---- scores: one fused 89-deep matmul per (g,h), exp ----
                attn = sp.tile([119, 57 * 128], f16, tag="attn")
                for pk in range(15):         # packs of 4 (g,h) tiles; 57=14*4+1
                    n = min(4, 57 - pk * 4)
                    pd = ps.tile([128, 512], f32, tag="ps32")
                    for u in range(n):
                        idx = pk * 4 + u
                        g, h = divmod(idx, H)
                        t0, kn, qn = _gdims(g)
                        ke = min(t0 + 128, TC)   # pad keys to 128 cols (FWL)
                        o = u * 128
                        nc.tensor.matmul(pd[0:ke - t0, o:o + qn],
                                         kf[h][:, t0:t0 + (ke - t0)],
                                         qf[h][:, t0:t0 + qn],
                                         start=True, stop=True)
                    nc.scalar.activation(attn[:, pk * 512:pk * 512 + n * 128],
                                         pd[0:119, 0:n * 128], EXP)

                # ---- V projection (overlapped 128-token groups) ----
                vt = sp.tile([119, G * 195], f16, tag="vt")
                nc.gpsimd.memset(
                    vt[:].rearrange("p (g hh c) -> p g hh c", hh=3, c=65)[:, :, :, 64:65],
                    1.0)
                for gp in range(10):         # packs of 2 groups, 2 banks
                    n = min(2, G - gp * 2)
                    pvs = []
                    for _ in range(n):
                        pv = ps.tile([128, 512], f32, tag="ps32", name="pv")
                        pvs.append(pv)
                    gks = []
                    for u in range(n):
                        t0, kn, qn = _gdims(gp * 2 + u)
                        gks.append((slice(t0, t0 + min(t0 + 128, TC) - t0), kn))
                    for u in range(n):
                        t0, kn = gks[u][0].start, gks[u][1]
                        sk = slice(t0, min(t0 + 128, TC))
                        nc.tensor.matmul(pvs[u][0:sk.stop - sk.start, 0:192],
                                         xt0[:, sk], wv0[:], start=True, stop=False)
                    for u in range(n):
                        t0, kn = gks[u][0].start, gks[u][1]
                        sk = slice(t0, min(t0 + 128, TC))
                        nc.tensor.matmul(pvs[u][0:sk.stop - sk.start, 0:192],
                                         xt1[:, sk], wv1[:], start=False, stop=True)
                    g0 = gp * 2
                    for u in range(n):
                        srcv = pvs[u][:, 0:192].rearrange("p (hh c) -> p hh c", c=64)[0:119]
                        dst = vt[:].rearrange("p (g hh c) -> p g hh c", hh=3, c=65)[
                            :, g0 + u, :, 0:64]
                        if (gp + u) % 2 == 0:
                            nc.vector.tensor_copy(dst, srcv)
                        else:
                            nc.scalar.copy(dst, srcv)

                # ---- attention @ V (+denominator), normalize via TT ----
                avout = sp.tile([119, G * 192], f16, tag="avout")
                recip = sp.tile([119, G * H], f32, tag="recip")
                vtv = vt[:].rearrange("p (g c) -> p g c", c=195)
                for gp in range(10):         # packs of 2 groups
                    n = min(2, G - gp * 2)
                    pa = ps.tile([128, 390], f32, tag="ps32")
                    for u in range(n):
                        g = gp * 2 + u
                        t0, kn, qn = _gdims(g)
                        qe = 128 if g < 18 else qn   # pad queries (FWL)
                        for h in range(H):
                            idx = g * H + h
                            nc.tensor.matmul(
                                pa[0:qe, u * 195 + 65 * h:u * 195 + 65 * h + 65],
                                attn[0:kn, idx * 128:idx * 128 + qe],
                                vtv[0:kn, g, 65 * h:65 * h + 65],
                                start=True, stop=True)
                    g0 = gp * 2
                    pav = pa[0:119].rearrange("p (g hh c) -> p g hh c", hh=3, c=65)
                    nc.vector.reciprocal(
                        recip[:, g0 * H:(g0 + n) * H].rearrange(
                            "p (g hh) -> p g hh", hh=3),
                        pav[:, 0:n, :, 64])
                    rb = recip[:, g0 * H:(g0 + n) * H].rearrange(
                        "p (g hh) -> p g hh", hh=3).unsqueeze(3).broadcast_to(
                        (119, n, 3, 64))
                    nc.vector.tensor_mul(
                        avout[:].rearrange("p (g hh c) -> p g hh c", hh=3, c=64)[
                            :, g0:g0 + n],
                        pav[:, 0:n, :, 0:64], rb)

                # ---- transpose avout to feature-major ----
                aot0 = sp.tile([128, G * 128], f16, tag="aot0")
                aot1 = sp.tile([64, G * 128], f16, tag="aot1")
                for gp in range(5):          # packs of 4 groups
                    n = min(4, G - gp * 4)
                    pta = pst.tile([128, 512], f16, tag="pst")
                    ptb = pst.tile([64, 512], f16, tag="pstb")
                    for u in range(n):
                        g = gp * 4 + u
                        t0, kn, qn = _gdims(g)
                        nc.tensor.transpose(pta[:, u * 128:u * 128 + qn],
                                            avout[0:qn, g * 192:g * 192 + 128],
                                            ident[0:qn, 0:qn])
                        nc.tensor.transpose(ptb[:, u * 128:u * 128 + qn],
                                            avout[0:qn, g * 192 + 128:g * 192 + 192],
                                            ident[0:qn, 0:qn])
                    cs = slice(gp * 512, gp * 512 + n * 128)
                    nc.vector.tensor_copy(aot0[:, cs], pta[:, 0:n * 128])
                    nc.scalar.copy(aot1[:, cs], ptb[:, 0:n * 128])

                # ---- output projection ----
                fin = sp.tile([119, G * 192], f16, tag="fin")
                for gp in range(10):         # packs of 2 groups, 2 banks
                    n = min(2, G - gp * 2)
                    pos_ = []
                    for _ in range(n):
                        po = ps.tile([128, 512], f32, tag="ps32", name="po")
                        pos_.append(po)
                    for u in range(n):
                        g = gp * 2 + u
                        nc.tensor.matmul(pos_[u][:, 0:192],
                                         aot0[:, g * 128:(g + 1) * 128],
                                         wout0[:], start=True, stop=False)
                    for u in range(n):
                        g = gp * 2 + u
                        nc.tensor.matmul(pos_[u][:, 0:192],
                                         aot1[:, g * 128:(g + 1) * 128],
                                         wout1[:], start=False, stop=True)
                    g0 = gp * 2
                    for u in range(n):
                        dst = fin[:, (g0 + u) * 192:(g0 + u + 1) * 192]
                        if (gp + u) % 2 == 0:
                            nc.scalar.copy(dst, pos_[u][0:119, 0:192])
                        else:
                            nc.vector.tensor_copy(dst, pos_[u][0:119, 0:192])

                # ---- store ----
                nc.sync.dma_start(
                    out=y_out[r0:r0 + 18 * GS, :].rearrange("(g p) d -> p g d", p=GS),
                    in_=fin[:].rearrange("p (g d) -> p g d", d=192)[:, 0:18, :])
                nc.sync.dma_start(
                    out=y_out[r0 + 18 * GS:r0 + TC, :],
                    in_=fin[0:34, 18 * 192:19 * 192])

    nc.finalize()
    return nc


def kernel(x, W_qkv, b_qkv, key_rel, key_rel_diag, W_out, b_out):
    from concourse.bass_utils import run_bass_kernel_spmd

    # pre-transpose per chunk on host: (core, chunk, TC, DIM) -> (core, chunk, DIM, TC)
    xt = np.ascontiguousarray(
        np.asarray(x, dtype=np.float16).reshape(NCORES, NCHUNK, TC, DIM)
        .transpose(0, 1, 3, 2)).reshape(NCORES, NCHUNK * DIM, TC)
    consts = _build_host_constants(
        np.asarray(W_qkv, np.float32), np.asarray(b_qkv, np.float32),
        np.asarray(key_rel, np.float32), np.asarray(key_rel_diag, np.float32),
        np.asarray(W_out, np.float32), np.asarray(b_out, np.float32))

    if "nc" not in _CACHED:
        _CACHED["nc"] = _build_bass()
    nc = _CACHED["nc"]

    in_maps = [dict(consts, x=xt[k]) for k in range(NCORES)]
    res = run_bass_kernel_spmd(nc, in_maps, core_ids=list(range(NCORES)))
    _CACHED["last_result"] = res
    out = np.stack([res.results[k]["y"] for k in range(NCORES)], axis=0)
    return out.reshape(B, V, DIM).astype(np.float32)
